# revision 41
# baseline (speedup 1.0000x reference)
"""Mesh chamfer/normal/edge loss on 8 Trainium2 NeuronCores.

Sharding: data-parallel over the 4 meshes x 2 row-halves -> 8 cores.
Each core computes its 2048x4096 squared-distance matrix on-device
(single K=5 matmul per 512-col chunk: d2 = x2 + y2 - 2 x.y).

Each 128-row x 2048-col d2 tile is cast PSUM->SBUF bf16 by the scalar
engine (or the vector engine for some tiles, balancing the two), then
either:
  - 'F' fold units: DVE min-accumulates the tile into a column-min
    accumulator and folds rows to [128,1024] width-2 pair-mins (host
    recomputes the winning pair's 2 columns exactly), or
  - '0' ship units: the raw bf16 tile goes to DRAM and the host does
    that tile's row- and column-reductions itself (the DMA queues have
    slack; GPSIMD can't help - neuronxcc rejects two-tensor Pool ops).
"""

import os
import sys

for _p in ("/opt/trn_rl_repo", "/root/.axon_site/_ro/trn_rl_repo"):
    if os.path.isdir(_p) and _p not in sys.path:
        sys.path.append(_p)

import numpy as np

# ---------------- problem constants (hardcoded) ----------------
B = 4
NSAMP = 4096          # sampled points per mesh (both pred and gt)
ROWS_PER_CORE = 2048  # pred rows per core (half a mesh)
T_TILES = 16          # row tiles of 128
MCOLS = 4096          # gt points per mesh
HALF = 2048           # column half processed per tile
N_UNITS = 32          # (2 col-halves) x (16 row-tiles)
N_CORES = 8

CHAMFER_W = 1.0
NORM_W = 0.1
EDGE_W = 0.5
EPS = 1e-12

# per-unit schedule: (feed, kind)
#  feed 'A': scalar engine casts PSUM f32 -> SBUF bf16
#  feed 'V': vector engine casts (tensor_scalar_max, clamps at 0)
#  kind 'F': fold unit - DVE colmin into the accumulator chain + one DVE
#            fold to [128,1024] pair-mins; host recomputes the 2 candidates
#  kind '0': ship unit - the raw bf16 tile goes to DRAM; the host does both
#            its row-side and column-side reductions
# (GPSIMD takes no part: neuronxcc rejects every two-tensor op on Pool.)
PLAN_HALF = [
    ("A", "F"),   # t0 seeds the accumulator (feed writes acc directly)
    ("A", "0"), ("V", "0"), ("A", "F"), ("A", "0"), ("V", "0"),
    ("A", "F"), ("A", "0"), ("V", "0"), ("A", "F"), ("A", "0"),
    ("A", "F"), ("V", "0"), ("A", "F"), ("V", "0"), ("A", "0"),
]
PLAN = PLAN_HALF + PLAN_HALF

# ---------------- bass program (built once) ----------------
_COMPILED = {}


def build_bass():
    import concourse.bacc as bacc
    import concourse.mybir as mybir
    import concourse.tile as tile

    f32 = mybir.dt.float32
    f32r = mybir.dt.float32r
    bf16 = mybir.dt.bfloat16
    amin = mybir.AluOpType.min

    nc = bacc.Bacc("TRN2", target_bir_lowering=False, debug=False)

    lhsT_d = nc.dram_tensor("lhsT", [5, ROWS_PER_CORE], f32r, kind="ExternalInput")
    rhs_d = nc.dram_tensor("rhs", [5, MCOLS], f32r, kind="ExternalInput")
    cm_d = nc.dram_tensor("cm", [N_UNITS, 128, HALF], bf16, kind="ExternalOutput")
    accD_d = nc.dram_tensor("accD", [128, MCOLS], bf16, kind="ExternalOutput")

    with tile.TileContext(nc) as tc:
        with (
            tc.tile_pool(name="ops", bufs=1) as ops_pool,
            tc.tile_pool(name="scopy", bufs=16) as s_pool,
            tc.tile_pool(name="cms", bufs=8) as cm_pool,
            tc.tile_pool(name="psum", bufs=2, space="PSUM") as psum_pool,
        ):
            lhsT_sb = ops_pool.tile([5, ROWS_PER_CORE], f32r)
            rhs_sb = ops_pool.tile([5, MCOLS], f32r)
            # input loads on the SP queue, head chunks first so the first
            # matmuls can start while the rest streams in
            nc.sync.dma_start(lhsT_sb[:, 0:128], lhsT_d[:, 0:128])
            nc.sync.dma_start(rhs_sb[:, 0:2048], rhs_d[:, 0:2048])
            nc.sync.dma_start(lhsT_sb[:, 128:2048], lhsT_d[:, 128:2048])
            nc.sync.dma_start(rhs_sb[:, 2048:4096], rhs_d[:, 2048:4096])

            accD = ops_pool.tile([128, MCOLS], bf16)

            # last fold unit per half -> ship that acc half right after it
            last_fold = {}
            for u, (feed, kind) in enumerate(PLAN):
                if kind == "F":
                    last_fold[u // T_TILES] = u

            def emit_reduction(u, s, do_colmin):
                """colmin + rowmin + stores for unit u (s = its bf16 tile)."""
                half = u // T_TILES
                feed, kind = PLAN[u]
                accD_h = accD[:, half * HALF:(half + 1) * HALF]
                if kind == "F":
                    if do_colmin:
                        nc.vector.tensor_tensor(accD_h, accD_h, s, op=amin)
                    cmq = cm_pool.tile([128, 1024], bf16, tag="cmq")
                    nc.vector.tensor_tensor(
                        cmq[:], s[:, 0:1024], s[:, 1024:2048], op=amin)
                    nc.sync.dma_start(cm_d[u, :, 0:1024], cmq[:])
                else:
                    nc.sync.dma_start(cm_d[u], s)
                if last_fold.get(half) == u:
                    nc.sync.dma_start(
                        accD_d[:, half * HALF:(half + 1) * HALF], accD_h)

            # software-pipelined emission: feeds go out immediately (so PSUM
            # recycles at feed pace and PE never starves); the reduction work
            # for a unit trails LAG units behind in each engine's stream
            LAG = 2
            seeded = {}
            pending = []
            for u in range(N_UNITS):
                half, t = divmod(u, T_TILES)
                feed, kind = PLAN[u]
                accD_h = accD[:, half * HALF:(half + 1) * HALF]

                ps = psum_pool.tile([128, HALF], f32)
                for j in range(4):
                    nc.tensor.matmul(
                        ps[:, j * 512:(j + 1) * 512],
                        lhsT_sb[:, t * 128:(t + 1) * 128],
                        rhs_sb[:, half * HALF + j * 512:
                               half * HALF + (j + 1) * 512],
                        start=True,
                        stop=True,
                    )

                if kind == "F" and not seeded.get(half):
                    # chain seed: the feed writes the accumulator itself;
                    # this unit's fold later reads acc_h directly
                    seeded[half] = True
                    s = accD_h
                    do_colmin = False
                else:
                    s_tile = s_pool.tile([128, HALF], bf16, tag="scp")
                    s = s_tile[:]
                    do_colmin = True
                if feed == "A":
                    nc.scalar.copy(s, ps[:])
                else:
                    nc.vector.tensor_scalar_max(s, ps[:], 0.0)

                pending.append((u, s, do_colmin))
                if len(pending) > LAG:
                    emit_reduction(*pending.pop(0))
            while pending:
                emit_reduction(*pending.pop(0))

    nc.compile()
    return nc


def _get_nc():
    if "nc" not in _COMPILED:
        _COMPILED["nc"] = build_bass()
    return _COMPILED["nc"]


# ---------------- host-side sampling (exact replica of reference) ----------------

def _sample_meshes(predicted_vertices, predicted_faces, gt_vertices, gt_faces):
    import jax
    import jax.numpy as jnp

    cpu = jax.devices("cpu")[0]

    def face_geometry(vertices, faces):
        v0 = vertices[:, faces[:, 0]]
        v1 = vertices[:, faces[:, 1]]
        v2 = vertices[:, faces[:, 2]]
        cross = jnp.cross(v1 - v0, v2 - v0)
        area2 = jnp.linalg.norm(cross, axis=-1)
        normals = cross / (area2[..., None] + EPS)
        return v0, v1, v2, 0.5 * area2, normals

    def sample_points(vertices, faces, n_samples, key):
        Bb = vertices.shape[0]
        v0, v1, v2, area, normals = face_geometry(vertices, faces)
        k_face, k_u, k_v = jax.random.split(key, 3)
        logits = jnp.log(area + EPS)
        face_idx = jax.random.categorical(
            k_face, logits[:, None, :], axis=-1, shape=(Bb, n_samples)
        )
        gather = lambda a: jnp.take_along_axis(a, face_idx[..., None], axis=1)
        p0, p1, p2 = gather(v0), gather(v1), gather(v2)
        u = jax.random.uniform(k_u, (Bb, n_samples, 1))
        v = jax.random.uniform(k_v, (Bb, n_samples, 1))
        r1 = jnp.sqrt(u)
        points = (1.0 - r1) * p0 + r1 * (1.0 - v) * p1 + r1 * v * p2
        point_normals = gather(normals)
        return points, point_normals

    def sample_all(pv, pf, gv, gf):
        key = jax.random.key(42)
        kp, kg = jax.random.split(key)
        pred_pts, pred_nrm = sample_points(pv, pf, NSAMP, kp)
        gt_pts, gt_nrm = sample_points(gv, gf, NSAMP, kg)
        return pred_pts, pred_nrm, gt_pts, gt_nrm

    fn = _COMPILED.get("sample_jit")
    if fn is None:
        fn = jax.jit(sample_all, backend="cpu")
        _COMPILED["sample_jit"] = fn

    with jax.default_device(cpu):
        out = fn(
            jnp.asarray(predicted_vertices), jnp.asarray(predicted_faces),
            jnp.asarray(gt_vertices), jnp.asarray(gt_faces),
        )
        out = tuple(np.asarray(a) for a in out)
    return out


# ---------------- main entry ----------------

def kernel(predicted_vertices, predicted_faces, gt_vertices, gt_faces):
    from concourse.bass_utils import run_bass_kernel_spmd

    predicted_vertices = np.asarray(predicted_vertices, dtype=np.float32)
    gt_vertices = np.asarray(gt_vertices, dtype=np.float32)

    pred_pts, pred_nrm, gt_pts, gt_nrm = _sample_meshes(
        predicted_vertices, predicted_faces, gt_vertices, gt_faces
    )

    # per-core operands: core c -> mesh b = c//2, row half rh = c%2
    x2_all = np.sum(pred_pts * pred_pts, axis=-1)  # [B, 4096]
    y2_all = np.sum(gt_pts * gt_pts, axis=-1)      # [B, 4096]

    in_maps = []
    for c in range(N_CORES):
        b, rh = divmod(c, 2)
        x = pred_pts[b, rh * ROWS_PER_CORE:(rh + 1) * ROWS_PER_CORE]
        y = gt_pts[b]
        x2 = x2_all[b, rh * ROWS_PER_CORE:(rh + 1) * ROWS_PER_CORE]
        y2 = y2_all[b]
        lhsT = np.empty((5, ROWS_PER_CORE), np.float32)
        lhsT[0:3] = -2.0 * x.T
        lhsT[3] = x2
        lhsT[4] = 1.0
        rhs = np.empty((5, MCOLS), np.float32)
        rhs[0:3] = y.T
        rhs[3] = 1.0
        rhs[4] = y2
        in_maps.append({"lhsT": lhsT, "rhs": rhs})

    nc = _get_nc()
    res = run_bass_kernel_spmd(nc, in_maps, list(range(N_CORES))).results

    # ---------------- host postprocessing ----------------
    min_x2y = np.empty((B, NSAMP), np.float32)
    idx_p2g = np.empty((B, NSAMP), np.int64)
    min_y2x = np.empty((B, MCOLS), np.float32)

    rows_l = np.arange(ROWS_PER_CORE)
    # per unit: raw-shipped tile ('0') vs on-device width-2 pair-mins ('F')
    shipped = np.array([PLAN[u][1] == "0" for u in range(N_UNITS)])
    for b in range(B):
        col_partials = []
        for rh in range(2):
            r = res[2 * b + rh]
            cm = np.asarray(r["cm"], np.float32)           # [32, 128, 2048]
            # device column partial (fold units' rows) + shipped tiles
            aD = np.asarray(r["accD"], np.float32)         # [128, 4096]
            colmin = aD.min(axis=0)                        # [4096]
            # reduce each unit to (best value, candidate col pair) per row
            bv = np.empty((2, T_TILES, 128), np.float32)
            bc = np.empty((2, T_TILES, 128, 2), np.int64)
            for u in range(N_UNITS):
                h, t = divmod(u, T_TILES)
                base = h * HALF
                if shipped[u]:
                    j = np.argmin(cm[u], axis=1)           # [128]
                    bv[h, t] = cm[u][np.arange(128), j]
                    bc[h, t, :, 0] = base + j
                    bc[h, t, :, 1] = base + j
                    np.minimum(colmin[base:base + HALF], cm[u].min(axis=0),
                               out=colmin[base:base + HALF])
                else:
                    M = cm[u, :, 0:1024]
                    j = np.argmin(M, axis=1)
                    bv[h, t] = M[np.arange(128), j]
                    bc[h, t, :, 0] = base + j
                    bc[h, t, :, 1] = base + j + 1024
            col_partials.append(colmin)
            # pick the winning half per row, recompute its <=2 candidates
            hwin = np.argmin(bv, axis=0).reshape(ROWS_PER_CORE)   # [2048]
            cand = bc.transpose(1, 2, 0, 3).reshape(ROWS_PER_CORE, 2, 2)
            cand = cand[rows_l, hwin]                      # [2048, 2]

            xb = pred_pts[b, rh * ROWS_PER_CORE:(rh + 1) * ROWS_PER_CORE]
            ycand = gt_pts[b][cand]                        # [2048, 2, 3]
            d2c = (
                x2_all[b, rh * ROWS_PER_CORE:(rh + 1) * ROWS_PER_CORE][:, None]
                + y2_all[b][cand]
                - 2.0 * np.einsum("nd,nkd->nk", xb, ycand, dtype=np.float32)
            ).astype(np.float32)
            d2c = np.maximum(d2c, 0.0)
            within = np.argmin(d2c, axis=1)
            sl = slice(rh * ROWS_PER_CORE, (rh + 1) * ROWS_PER_CORE)
            min_x2y[b, sl] = d2c[rows_l, within]
            idx_p2g[b, sl] = cand[rows_l, within]

        acc_b = np.minimum(col_partials[0], col_partials[1])
        min_y2x[b] = np.maximum(acc_b, 0.0)

    chamfer = np.float32(np.mean(min_x2y)) + np.float32(np.mean(min_y2x))

    # normal consistency
    matched = np.take_along_axis(gt_nrm, idx_p2g[..., None], axis=1)
    cos = np.abs(np.sum(pred_nrm * matched, axis=-1))
    normal_loss = np.float32(np.mean(1.0 - cos))

    # edge loss (exact, on host)
    pf = np.asarray(predicted_faces).astype(np.int64)
    v0 = predicted_vertices[:, pf[:, 0]]
    v1 = predicted_vertices[:, pf[:, 1]]
    v2 = predicted_vertices[:, pf[:, 2]]
    e = np.concatenate([v1 - v0, v2 - v1, v0 - v2], axis=1)
    edge_loss = np.float32(np.mean(np.sum(e * e, axis=-1)))

    total = (
        np.float32(CHAMFER_W) * chamfer
        + np.float32(NORM_W) * normal_loss
        + np.float32(EDGE_W) * edge_loss
    )
    return np.asarray(total, dtype=np.float32)


# revision 46
# speedup vs baseline: 1.1267x; 1.1267x over previous
"""Mesh chamfer/normal/edge loss on 8 Trainium2 NeuronCores.

Sharding: data-parallel over the 4 meshes x 2 row-halves -> 8 cores.
Each core computes its 2048x4096 squared-distance matrix on-device
(single K=5 matmul per 512-col chunk: d2 = x2 + y2 - 2 x.y).

Each 128-row x 2048-col d2 tile is cast PSUM->SBUF bf16 by the scalar
engine (or the vector engine for some tiles, balancing the two), then
either:
  - 'F' fold units: DVE min-accumulates the tile into a column-min
    accumulator and folds rows to [128,1024] width-2 pair-mins (host
    recomputes the winning pair's 2 columns exactly), or
  - '0' ship units: the raw bf16 tile goes to DRAM and the host does
    that tile's row- and column-reductions itself (the DMA queues have
    slack; GPSIMD can't help - neuronxcc rejects two-tensor Pool ops).
"""

import os
import sys

for _p in ("/opt/trn_rl_repo", "/root/.axon_site/_ro/trn_rl_repo"):
    if os.path.isdir(_p) and _p not in sys.path:
        sys.path.append(_p)

import numpy as np

# ---------------- problem constants (hardcoded) ----------------
B = 4
NSAMP = 4096          # sampled points per mesh (both pred and gt)
ROWS_PER_CORE = 2048  # pred rows per core (half a mesh)
T_TILES = 16          # row tiles of 128
MCOLS = 4096          # gt points per mesh
HALF = 2048           # column half processed per tile
N_UNITS = 32          # (2 col-halves) x (16 row-tiles)
N_CORES = 8

CHAMFER_W = 1.0
NORM_W = 0.1
EDGE_W = 0.5
EPS = 1e-12

# per-unit schedule: (feed, kind)
#  feed 'A': scalar engine casts PSUM f32 -> SBUF bf16
#  feed 'V': vector engine casts (tensor_scalar_max, clamps at 0)
#  kind 'F': fold unit - DVE colmin into the accumulator chain + one DVE
#            fold to [128,1024] pair-mins; host recomputes the 2 candidates
#  kind '0': ship unit - the raw bf16 tile goes to DRAM; the host does both
#            its row-side and column-side reductions
# (GPSIMD takes no part: neuronxcc rejects every two-tensor op on Pool.)
PLAN_HALF = [
    ("A", "F"),   # t0 seeds the accumulator (feed writes acc directly)
    ("A", "0"), ("V", "0"), ("A", "F"), ("A", "0"), ("V", "0"),
    ("A", "F"), ("A", "0"), ("V", "0"), ("A", "F"), ("A", "0"),
    ("A", "F"), ("V", "0"), ("A", "F"), ("V", "0"), ("A", "0"),
]
PLAN = PLAN_HALF + PLAN_HALF

# ---------------- bass program (built once) ----------------
_COMPILED = {}


def build_bass():
    import concourse.bacc as bacc
    import concourse.mybir as mybir
    import concourse.tile as tile

    f32 = mybir.dt.float32
    f32r = mybir.dt.float32r
    bf16 = mybir.dt.bfloat16
    amin = mybir.AluOpType.min

    nc = bacc.Bacc("TRN2", target_bir_lowering=False, debug=False)

    lhsT_d = nc.dram_tensor("lhsT", [5, ROWS_PER_CORE], f32r, kind="ExternalInput")
    rhs_d = nc.dram_tensor("rhs", [5, MCOLS], f32r, kind="ExternalInput")
    cm_d = nc.dram_tensor("cm", [N_UNITS, 128, HALF], bf16, kind="ExternalOutput")
    accD_d = nc.dram_tensor("accD", [128, MCOLS], bf16, kind="ExternalOutput")

    with tile.TileContext(nc) as tc:
        with (
            tc.tile_pool(name="ops", bufs=1) as ops_pool,
            tc.tile_pool(name="scopy", bufs=16) as s_pool,
            tc.tile_pool(name="cms", bufs=8) as cm_pool,
            tc.tile_pool(name="psum", bufs=4, space="PSUM") as psum_pool,
        ):
            lhsT_sb = ops_pool.tile([5, ROWS_PER_CORE], f32r)
            rhs_sb = ops_pool.tile([5, MCOLS], f32r)
            # input loads on the SP queue, head chunks first so the first
            # matmuls can start while the rest streams in
            nc.sync.dma_start(lhsT_sb[:, 0:128], lhsT_d[:, 0:128])
            nc.sync.dma_start(rhs_sb[:, 0:2048], rhs_d[:, 0:2048])
            nc.sync.dma_start(lhsT_sb[:, 128:2048], lhsT_d[:, 128:2048])
            nc.sync.dma_start(rhs_sb[:, 2048:4096], rhs_d[:, 2048:4096])

            accD = ops_pool.tile([128, MCOLS], bf16)

            # last fold unit per half -> ship that acc half right after it
            last_fold = {}
            for u, (feed, kind) in enumerate(PLAN):
                if kind == "F":
                    last_fold[u // T_TILES] = u

            def emit_reduction(u, s, do_colmin):
                """colmin + rowmin + stores for unit u (s = its bf16 tile)."""
                half = u // T_TILES
                feed, kind = PLAN[u]
                accD_h = accD[:, half * HALF:(half + 1) * HALF]
                if kind == "F":
                    if do_colmin:
                        nc.vector.tensor_tensor(accD_h, accD_h, s, op=amin)
                    cmq = cm_pool.tile([128, 1024], bf16, tag="cmq")
                    nc.vector.tensor_tensor(
                        cmq[:], s[:, 0:1024], s[:, 1024:2048], op=amin)
                    nc.sync.dma_start(cm_d[u, :, 0:1024], cmq[:])
                else:
                    nc.sync.dma_start(cm_d[u], s)
                if last_fold.get(half) == u:
                    nc.sync.dma_start(
                        accD_d[:, half * HALF:(half + 1) * HALF], accD_h)

            # software-pipelined emission: feeds go out immediately (so PSUM
            # recycles at feed pace and PE never starves); the reduction work
            # for a unit trails LAG units behind in each engine's stream
            LAG = 2
            seeded = {}
            pending = []
            for u in range(N_UNITS):
                half, t = divmod(u, T_TILES)
                feed, kind = PLAN[u]
                accD_h = accD[:, half * HALF:(half + 1) * HALF]

                # two [128,1024] psum tiles per unit (4 rotating slots in
                # the pool): each half is drained right after its 2 matmuls,
                # so PE is never gated on a whole-tile drain - the 2-slot
                # ping-pong was the pipeline's pacing item
                ps_parts = []
                for p in range(2):
                    psq = psum_pool.tile([128, 1024], f32, tag="psq")
                    ps_parts.append(psq)
                    for j in range(2):
                        c0 = half * HALF + p * 1024 + j * 512
                        nc.tensor.matmul(
                            psq[:, j * 512:(j + 1) * 512],
                            lhsT_sb[:, t * 128:(t + 1) * 128],
                            rhs_sb[:, c0:c0 + 512],
                            start=True,
                            stop=True,
                        )

                if kind == "F" and not seeded.get(half):
                    # chain seed: the feed writes the accumulator itself;
                    # this unit's fold later reads acc_h directly
                    seeded[half] = True
                    s = accD_h
                    do_colmin = False
                else:
                    s_tile = s_pool.tile([128, HALF], bf16, tag="scp")
                    s = s_tile[:]
                    do_colmin = True
                for p in range(2):
                    dst = s[:, p * 1024:(p + 1) * 1024]
                    use_act = feed == "A" or (feed == "M" and p == 0)
                    if use_act:
                        nc.scalar.copy(dst, ps_parts[p][:])
                    else:
                        nc.vector.tensor_scalar_max(dst, ps_parts[p][:], 0.0)

                pending.append((u, s, do_colmin))
                if len(pending) > LAG:
                    emit_reduction(*pending.pop(0))
            while pending:
                emit_reduction(*pending.pop(0))

    nc.compile()
    return nc


def _get_nc():
    if "nc" not in _COMPILED:
        _COMPILED["nc"] = build_bass()
    return _COMPILED["nc"]


# ---------------- host-side sampling (exact replica of reference) ----------------

def _sample_meshes(predicted_vertices, predicted_faces, gt_vertices, gt_faces):
    import jax
    import jax.numpy as jnp

    cpu = jax.devices("cpu")[0]

    def face_geometry(vertices, faces):
        v0 = vertices[:, faces[:, 0]]
        v1 = vertices[:, faces[:, 1]]
        v2 = vertices[:, faces[:, 2]]
        cross = jnp.cross(v1 - v0, v2 - v0)
        area2 = jnp.linalg.norm(cross, axis=-1)
        normals = cross / (area2[..., None] + EPS)
        return v0, v1, v2, 0.5 * area2, normals

    def sample_points(vertices, faces, n_samples, key):
        Bb = vertices.shape[0]
        v0, v1, v2, area, normals = face_geometry(vertices, faces)
        k_face, k_u, k_v = jax.random.split(key, 3)
        logits = jnp.log(area + EPS)
        face_idx = jax.random.categorical(
            k_face, logits[:, None, :], axis=-1, shape=(Bb, n_samples)
        )
        gather = lambda a: jnp.take_along_axis(a, face_idx[..., None], axis=1)
        p0, p1, p2 = gather(v0), gather(v1), gather(v2)
        u = jax.random.uniform(k_u, (Bb, n_samples, 1))
        v = jax.random.uniform(k_v, (Bb, n_samples, 1))
        r1 = jnp.sqrt(u)
        points = (1.0 - r1) * p0 + r1 * (1.0 - v) * p1 + r1 * v * p2
        point_normals = gather(normals)
        return points, point_normals

    def sample_all(pv, pf, gv, gf):
        key = jax.random.key(42)
        kp, kg = jax.random.split(key)
        pred_pts, pred_nrm = sample_points(pv, pf, NSAMP, kp)
        gt_pts, gt_nrm = sample_points(gv, gf, NSAMP, kg)
        return pred_pts, pred_nrm, gt_pts, gt_nrm

    fn = _COMPILED.get("sample_jit")
    if fn is None:
        fn = jax.jit(sample_all, backend="cpu")
        _COMPILED["sample_jit"] = fn

    with jax.default_device(cpu):
        out = fn(
            jnp.asarray(predicted_vertices), jnp.asarray(predicted_faces),
            jnp.asarray(gt_vertices), jnp.asarray(gt_faces),
        )
        out = tuple(np.asarray(a) for a in out)
    return out


# ---------------- main entry ----------------

def kernel(predicted_vertices, predicted_faces, gt_vertices, gt_faces):
    from concourse.bass_utils import run_bass_kernel_spmd

    predicted_vertices = np.asarray(predicted_vertices, dtype=np.float32)
    gt_vertices = np.asarray(gt_vertices, dtype=np.float32)

    pred_pts, pred_nrm, gt_pts, gt_nrm = _sample_meshes(
        predicted_vertices, predicted_faces, gt_vertices, gt_faces
    )

    # per-core operands: core c -> mesh b = c//2, row half rh = c%2
    x2_all = np.sum(pred_pts * pred_pts, axis=-1)  # [B, 4096]
    y2_all = np.sum(gt_pts * gt_pts, axis=-1)      # [B, 4096]

    in_maps = []
    for c in range(N_CORES):
        b, rh = divmod(c, 2)
        x = pred_pts[b, rh * ROWS_PER_CORE:(rh + 1) * ROWS_PER_CORE]
        y = gt_pts[b]
        x2 = x2_all[b, rh * ROWS_PER_CORE:(rh + 1) * ROWS_PER_CORE]
        y2 = y2_all[b]
        lhsT = np.empty((5, ROWS_PER_CORE), np.float32)
        lhsT[0:3] = -2.0 * x.T
        lhsT[3] = x2
        lhsT[4] = 1.0
        rhs = np.empty((5, MCOLS), np.float32)
        rhs[0:3] = y.T
        rhs[3] = 1.0
        rhs[4] = y2
        in_maps.append({"lhsT": lhsT, "rhs": rhs})

    nc = _get_nc()
    res = run_bass_kernel_spmd(nc, in_maps, list(range(N_CORES))).results

    # ---------------- host postprocessing ----------------
    min_x2y = np.empty((B, NSAMP), np.float32)
    idx_p2g = np.empty((B, NSAMP), np.int64)
    min_y2x = np.empty((B, MCOLS), np.float32)

    rows_l = np.arange(ROWS_PER_CORE)
    # per unit: raw-shipped tile ('0') vs on-device width-2 pair-mins ('F')
    shipped = np.array([PLAN[u][1] == "0" for u in range(N_UNITS)])
    for b in range(B):
        col_partials = []
        for rh in range(2):
            r = res[2 * b + rh]
            cm = np.asarray(r["cm"], np.float32)           # [32, 128, 2048]
            # device column partial (fold units' rows) + shipped tiles
            aD = np.asarray(r["accD"], np.float32)         # [128, 4096]
            colmin = aD.min(axis=0)                        # [4096]
            # reduce each unit to (best value, candidate col pair) per row
            bv = np.empty((2, T_TILES, 128), np.float32)
            bc = np.empty((2, T_TILES, 128, 2), np.int64)
            for u in range(N_UNITS):
                h, t = divmod(u, T_TILES)
                base = h * HALF
                if shipped[u]:
                    j = np.argmin(cm[u], axis=1)           # [128]
                    bv[h, t] = cm[u][np.arange(128), j]
                    bc[h, t, :, 0] = base + j
                    bc[h, t, :, 1] = base + j
                    np.minimum(colmin[base:base + HALF], cm[u].min(axis=0),
                               out=colmin[base:base + HALF])
                else:
                    M = cm[u, :, 0:1024]
                    j = np.argmin(M, axis=1)
                    bv[h, t] = M[np.arange(128), j]
                    bc[h, t, :, 0] = base + j
                    bc[h, t, :, 1] = base + j + 1024
            col_partials.append(colmin)
            # pick the winning half per row, recompute its <=2 candidates
            hwin = np.argmin(bv, axis=0).reshape(ROWS_PER_CORE)   # [2048]
            cand = bc.transpose(1, 2, 0, 3).reshape(ROWS_PER_CORE, 2, 2)
            cand = cand[rows_l, hwin]                      # [2048, 2]

            xb = pred_pts[b, rh * ROWS_PER_CORE:(rh + 1) * ROWS_PER_CORE]
            ycand = gt_pts[b][cand]                        # [2048, 2, 3]
            d2c = (
                x2_all[b, rh * ROWS_PER_CORE:(rh + 1) * ROWS_PER_CORE][:, None]
                + y2_all[b][cand]
                - 2.0 * np.einsum("nd,nkd->nk", xb, ycand, dtype=np.float32)
            ).astype(np.float32)
            d2c = np.maximum(d2c, 0.0)
            within = np.argmin(d2c, axis=1)
            sl = slice(rh * ROWS_PER_CORE, (rh + 1) * ROWS_PER_CORE)
            min_x2y[b, sl] = d2c[rows_l, within]
            idx_p2g[b, sl] = cand[rows_l, within]

        acc_b = np.minimum(col_partials[0], col_partials[1])
        min_y2x[b] = np.maximum(acc_b, 0.0)

    chamfer = np.float32(np.mean(min_x2y)) + np.float32(np.mean(min_y2x))

    # normal consistency
    matched = np.take_along_axis(gt_nrm, idx_p2g[..., None], axis=1)
    cos = np.abs(np.sum(pred_nrm * matched, axis=-1))
    normal_loss = np.float32(np.mean(1.0 - cos))

    # edge loss (exact, on host)
    pf = np.asarray(predicted_faces).astype(np.int64)
    v0 = predicted_vertices[:, pf[:, 0]]
    v1 = predicted_vertices[:, pf[:, 1]]
    v2 = predicted_vertices[:, pf[:, 2]]
    e = np.concatenate([v1 - v0, v2 - v1, v0 - v2], axis=1)
    edge_loss = np.float32(np.mean(np.sum(e * e, axis=-1)))

    total = (
        np.float32(CHAMFER_W) * chamfer
        + np.float32(NORM_W) * normal_loss
        + np.float32(EDGE_W) * edge_loss
    )
    return np.asarray(total, dtype=np.float32)


# revision 49
# speedup vs baseline: 1.1354x; 1.0077x over previous
"""Mesh chamfer/normal/edge loss on 8 Trainium2 NeuronCores.

Sharding: data-parallel over the 4 meshes x 2 row-halves -> 8 cores.
Each core computes its 2048x4096 squared-distance matrix on-device
(single K=5 matmul per 512-col chunk: d2 = x2 + y2 - 2 x.y).

Each 128-row x 2048-col d2 tile is cast PSUM->SBUF bf16 by the scalar
engine (or the vector engine for some tiles, balancing the two), then
either:
  - 'F' fold units: DVE min-accumulates the tile into a column-min
    accumulator and folds rows to [128,1024] width-2 pair-mins (host
    recomputes the winning pair's 2 columns exactly), or
  - '0' ship units: the raw bf16 tile goes to DRAM and the host does
    that tile's row- and column-reductions itself (the DMA queues have
    slack; GPSIMD can't help - neuronxcc rejects two-tensor Pool ops).
"""

import os
import sys

for _p in ("/opt/trn_rl_repo", "/root/.axon_site/_ro/trn_rl_repo"):
    if os.path.isdir(_p) and _p not in sys.path:
        sys.path.append(_p)

import numpy as np

# ---------------- problem constants (hardcoded) ----------------
B = 4
NSAMP = 4096          # sampled points per mesh (both pred and gt)
ROWS_PER_CORE = 2048  # pred rows per core (half a mesh)
T_TILES = 16          # row tiles of 128
MCOLS = 4096          # gt points per mesh
HALF = 2048           # column half processed per tile
N_UNITS = 32          # (2 col-halves) x (16 row-tiles)
N_CORES = 8

CHAMFER_W = 1.0
NORM_W = 0.1
EDGE_W = 0.5
EPS = 1e-12

# per-unit schedule: (feed, kind)
#  feed 'A': scalar engine casts PSUM f32 -> SBUF bf16
#  feed 'V': vector engine casts (tensor_scalar_max, clamps at 0)
#  kind 'F': fold unit - DVE colmin into the accumulator chain + one DVE
#            fold to [128,1024] pair-mins; host recomputes the 2 candidates
#  kind '0': ship unit - the raw bf16 tile goes to DRAM; the host does both
#            its row-side and column-side reductions
# (GPSIMD takes no part: neuronxcc rejects every two-tensor op on Pool.)
PLAN_HALF = [
    ("A", "F"),   # t0 seeds the accumulator (feed writes acc directly)
    ("A", "0"), ("V", "0"), ("A", "F"), ("A", "0"), ("V", "0"),
    ("A", "F"), ("A", "0"), ("V", "0"), ("A", "F"), ("A", "0"),
    ("A", "F"), ("V", "0"), ("A", "F"), ("V", "0"), ("A", "0"),
]
PLAN = PLAN_HALF + PLAN_HALF

# ---------------- bass program (built once) ----------------
_COMPILED = {}


def build_bass():
    import concourse.bacc as bacc
    import concourse.mybir as mybir
    import concourse.tile as tile

    f32 = mybir.dt.float32
    f32r = mybir.dt.float32r
    bf16 = mybir.dt.bfloat16
    amin = mybir.AluOpType.min

    nc = bacc.Bacc("TRN2", target_bir_lowering=False, debug=False)

    lhsT_d = nc.dram_tensor("lhsT", [5, ROWS_PER_CORE], f32r, kind="ExternalInput")
    rhs_d = nc.dram_tensor("rhs", [5, MCOLS], f32r, kind="ExternalInput")
    cm_d = nc.dram_tensor("cm", [N_UNITS, 128, HALF], bf16, kind="ExternalOutput")
    accD_d = nc.dram_tensor("accD", [128, MCOLS], bf16, kind="ExternalOutput")

    with tile.TileContext(nc) as tc:
        with (
            tc.tile_pool(name="ops", bufs=1) as ops_pool,
            tc.tile_pool(name="scopy", bufs=16) as s_pool,
            tc.tile_pool(name="cms", bufs=8) as cm_pool,
            tc.tile_pool(name="psum", bufs=4, space="PSUM") as psum_pool,
        ):
            lhsT_sb = ops_pool.tile([5, ROWS_PER_CORE], f32r)
            rhs_sb = ops_pool.tile([5, MCOLS], f32r)
            # input loads on the SP queue, head chunks first so the first
            # matmuls can start while the rest streams in (HWDGE generates
            # descriptors at ~625ns/DMA, so the order here is the head)
            nc.sync.dma_start(lhsT_sb[:, 0:128], lhsT_d[:, 0:128])
            nc.sync.dma_start(rhs_sb[:, 0:1024], rhs_d[:, 0:1024])
            nc.sync.dma_start(rhs_sb[:, 1024:2048], rhs_d[:, 1024:2048])
            nc.sync.dma_start(lhsT_sb[:, 128:2048], lhsT_d[:, 128:2048])
            nc.sync.dma_start(rhs_sb[:, 2048:4096], rhs_d[:, 2048:4096])

            accD = ops_pool.tile([128, MCOLS], bf16)

            # last fold unit per half -> ship that acc half right after it
            last_fold = {}
            for u, (feed, kind) in enumerate(PLAN):
                if kind == "F":
                    last_fold[u // T_TILES] = u

            def emit_reduction(u, s, do_colmin):
                """colmin + rowmin + stores for unit u (s = its bf16 tile)."""
                half = u // T_TILES
                feed, kind = PLAN[u]
                accD_h = accD[:, half * HALF:(half + 1) * HALF]
                if kind == "F":
                    if do_colmin:
                        nc.vector.tensor_tensor(accD_h, accD_h, s, op=amin)
                    cmq = cm_pool.tile([128, 1024], bf16, tag="cmq")
                    nc.vector.tensor_tensor(
                        cmq[:], s[:, 0:1024], s[:, 1024:2048], op=amin)
                    nc.sync.dma_start(cm_d[u, :, 0:1024], cmq[:])
                else:
                    nc.sync.dma_start(cm_d[u], s)
                if last_fold.get(half) == u:
                    nc.sync.dma_start(
                        accD_d[:, half * HALF:(half + 1) * HALF], accD_h)

            # software-pipelined emission: feeds go out immediately (so PSUM
            # recycles at feed pace and PE never starves); the reduction work
            # for a unit trails LAG units behind in each engine's stream
            LAG = 2
            seeded = {}
            pending = []
            for u in range(N_UNITS):
                half, t = divmod(u, T_TILES)
                feed, kind = PLAN[u]
                accD_h = accD[:, half * HALF:(half + 1) * HALF]

                # two [128,1024] psum tiles per unit (4 rotating slots in
                # the pool): each half is drained right after its 2 matmuls,
                # so PE is never gated on a whole-tile drain - the 2-slot
                # ping-pong was the pipeline's pacing item
                ps_parts = []
                for p in range(2):
                    psq = psum_pool.tile([128, 1024], f32, tag="psq")
                    ps_parts.append(psq)
                    for j in range(2):
                        c0 = half * HALF + p * 1024 + j * 512
                        nc.tensor.matmul(
                            psq[:, j * 512:(j + 1) * 512],
                            lhsT_sb[:, t * 128:(t + 1) * 128],
                            rhs_sb[:, c0:c0 + 512],
                            start=True,
                            stop=True,
                        )

                if kind == "F" and not seeded.get(half):
                    # chain seed: the feed writes the accumulator itself;
                    # this unit's fold later reads acc_h directly
                    seeded[half] = True
                    s = accD_h
                    do_colmin = False
                else:
                    s_tile = s_pool.tile([128, HALF], bf16, tag="scp")
                    s = s_tile[:]
                    do_colmin = True
                for p in range(2):
                    dst = s[:, p * 1024:(p + 1) * 1024]
                    use_act = feed == "A" or (feed == "M" and p == 0)
                    if use_act:
                        nc.scalar.copy(dst, ps_parts[p][:])
                    else:
                        nc.vector.tensor_scalar_max(dst, ps_parts[p][:], 0.0)

                pending.append((u, s, do_colmin))
                if len(pending) > LAG:
                    emit_reduction(*pending.pop(0))
            while pending:
                emit_reduction(*pending.pop(0))

    nc.compile()
    return nc


def _get_nc():
    if "nc" not in _COMPILED:
        _COMPILED["nc"] = build_bass()
    return _COMPILED["nc"]


# ---------------- host-side sampling (exact replica of reference) ----------------

def _sample_meshes(predicted_vertices, predicted_faces, gt_vertices, gt_faces):
    import jax
    import jax.numpy as jnp

    cpu = jax.devices("cpu")[0]

    def face_geometry(vertices, faces):
        v0 = vertices[:, faces[:, 0]]
        v1 = vertices[:, faces[:, 1]]
        v2 = vertices[:, faces[:, 2]]
        cross = jnp.cross(v1 - v0, v2 - v0)
        area2 = jnp.linalg.norm(cross, axis=-1)
        normals = cross / (area2[..., None] + EPS)
        return v0, v1, v2, 0.5 * area2, normals

    def sample_points(vertices, faces, n_samples, key):
        Bb = vertices.shape[0]
        v0, v1, v2, area, normals = face_geometry(vertices, faces)
        k_face, k_u, k_v = jax.random.split(key, 3)
        logits = jnp.log(area + EPS)
        face_idx = jax.random.categorical(
            k_face, logits[:, None, :], axis=-1, shape=(Bb, n_samples)
        )
        gather = lambda a: jnp.take_along_axis(a, face_idx[..., None], axis=1)
        p0, p1, p2 = gather(v0), gather(v1), gather(v2)
        u = jax.random.uniform(k_u, (Bb, n_samples, 1))
        v = jax.random.uniform(k_v, (Bb, n_samples, 1))
        r1 = jnp.sqrt(u)
        points = (1.0 - r1) * p0 + r1 * (1.0 - v) * p1 + r1 * v * p2
        point_normals = gather(normals)
        return points, point_normals

    def sample_all(pv, pf, gv, gf):
        key = jax.random.key(42)
        kp, kg = jax.random.split(key)
        pred_pts, pred_nrm = sample_points(pv, pf, NSAMP, kp)
        gt_pts, gt_nrm = sample_points(gv, gf, NSAMP, kg)
        return pred_pts, pred_nrm, gt_pts, gt_nrm

    fn = _COMPILED.get("sample_jit")
    if fn is None:
        fn = jax.jit(sample_all, backend="cpu")
        _COMPILED["sample_jit"] = fn

    with jax.default_device(cpu):
        out = fn(
            jnp.asarray(predicted_vertices), jnp.asarray(predicted_faces),
            jnp.asarray(gt_vertices), jnp.asarray(gt_faces),
        )
        out = tuple(np.asarray(a) for a in out)
    return out


# ---------------- main entry ----------------

def kernel(predicted_vertices, predicted_faces, gt_vertices, gt_faces):
    from concourse.bass_utils import run_bass_kernel_spmd

    predicted_vertices = np.asarray(predicted_vertices, dtype=np.float32)
    gt_vertices = np.asarray(gt_vertices, dtype=np.float32)

    pred_pts, pred_nrm, gt_pts, gt_nrm = _sample_meshes(
        predicted_vertices, predicted_faces, gt_vertices, gt_faces
    )

    # per-core operands: core c -> mesh b = c//2, row half rh = c%2
    x2_all = np.sum(pred_pts * pred_pts, axis=-1)  # [B, 4096]
    y2_all = np.sum(gt_pts * gt_pts, axis=-1)      # [B, 4096]

    in_maps = []
    for c in range(N_CORES):
        b, rh = divmod(c, 2)
        x = pred_pts[b, rh * ROWS_PER_CORE:(rh + 1) * ROWS_PER_CORE]
        y = gt_pts[b]
        x2 = x2_all[b, rh * ROWS_PER_CORE:(rh + 1) * ROWS_PER_CORE]
        y2 = y2_all[b]
        lhsT = np.empty((5, ROWS_PER_CORE), np.float32)
        lhsT[0:3] = -2.0 * x.T
        lhsT[3] = x2
        lhsT[4] = 1.0
        rhs = np.empty((5, MCOLS), np.float32)
        rhs[0:3] = y.T
        rhs[3] = 1.0
        rhs[4] = y2
        in_maps.append({"lhsT": lhsT, "rhs": rhs})

    nc = _get_nc()
    res = run_bass_kernel_spmd(nc, in_maps, list(range(N_CORES))).results

    # ---------------- host postprocessing ----------------
    min_x2y = np.empty((B, NSAMP), np.float32)
    idx_p2g = np.empty((B, NSAMP), np.int64)
    min_y2x = np.empty((B, MCOLS), np.float32)

    rows_l = np.arange(ROWS_PER_CORE)
    # per unit: raw-shipped tile ('0') vs on-device width-2 pair-mins ('F')
    shipped = np.array([PLAN[u][1] == "0" for u in range(N_UNITS)])
    for b in range(B):
        col_partials = []
        for rh in range(2):
            r = res[2 * b + rh]
            cm = np.asarray(r["cm"], np.float32)           # [32, 128, 2048]
            # device column partial (fold units' rows) + shipped tiles
            aD = np.asarray(r["accD"], np.float32)         # [128, 4096]
            colmin = aD.min(axis=0)                        # [4096]
            # reduce each unit to (best value, candidate col pair) per row
            bv = np.empty((2, T_TILES, 128), np.float32)
            bc = np.empty((2, T_TILES, 128, 2), np.int64)
            for u in range(N_UNITS):
                h, t = divmod(u, T_TILES)
                base = h * HALF
                if shipped[u]:
                    j = np.argmin(cm[u], axis=1)           # [128]
                    bv[h, t] = cm[u][np.arange(128), j]
                    bc[h, t, :, 0] = base + j
                    bc[h, t, :, 1] = base + j
                    np.minimum(colmin[base:base + HALF], cm[u].min(axis=0),
                               out=colmin[base:base + HALF])
                else:
                    M = cm[u, :, 0:1024]
                    j = np.argmin(M, axis=1)
                    bv[h, t] = M[np.arange(128), j]
                    bc[h, t, :, 0] = base + j
                    bc[h, t, :, 1] = base + j + 1024
            col_partials.append(colmin)
            # pick the winning half per row, recompute its <=2 candidates
            hwin = np.argmin(bv, axis=0).reshape(ROWS_PER_CORE)   # [2048]
            cand = bc.transpose(1, 2, 0, 3).reshape(ROWS_PER_CORE, 2, 2)
            cand = cand[rows_l, hwin]                      # [2048, 2]

            xb = pred_pts[b, rh * ROWS_PER_CORE:(rh + 1) * ROWS_PER_CORE]
            ycand = gt_pts[b][cand]                        # [2048, 2, 3]
            d2c = (
                x2_all[b, rh * ROWS_PER_CORE:(rh + 1) * ROWS_PER_CORE][:, None]
                + y2_all[b][cand]
                - 2.0 * np.einsum("nd,nkd->nk", xb, ycand, dtype=np.float32)
            ).astype(np.float32)
            d2c = np.maximum(d2c, 0.0)
            within = np.argmin(d2c, axis=1)
            sl = slice(rh * ROWS_PER_CORE, (rh + 1) * ROWS_PER_CORE)
            min_x2y[b, sl] = d2c[rows_l, within]
            idx_p2g[b, sl] = cand[rows_l, within]

        acc_b = np.minimum(col_partials[0], col_partials[1])
        min_y2x[b] = np.maximum(acc_b, 0.0)

    chamfer = np.float32(np.mean(min_x2y)) + np.float32(np.mean(min_y2x))

    # normal consistency
    matched = np.take_along_axis(gt_nrm, idx_p2g[..., None], axis=1)
    cos = np.abs(np.sum(pred_nrm * matched, axis=-1))
    normal_loss = np.float32(np.mean(1.0 - cos))

    # edge loss (exact, on host)
    pf = np.asarray(predicted_faces).astype(np.int64)
    v0 = predicted_vertices[:, pf[:, 0]]
    v1 = predicted_vertices[:, pf[:, 1]]
    v2 = predicted_vertices[:, pf[:, 2]]
    e = np.concatenate([v1 - v0, v2 - v1, v0 - v2], axis=1)
    edge_loss = np.float32(np.mean(np.sum(e * e, axis=-1)))

    total = (
        np.float32(CHAMFER_W) * chamfer
        + np.float32(NORM_W) * normal_loss
        + np.float32(EDGE_W) * edge_loss
    )
    return np.asarray(total, dtype=np.float32)


# revision 50
# speedup vs baseline: 1.1437x; 1.0073x over previous
"""Mesh chamfer/normal/edge loss on 8 Trainium2 NeuronCores.

Sharding: data-parallel over the 4 meshes x 2 row-halves -> 8 cores.
Each core computes its 2048x4096 squared-distance matrix on-device
(single K=5 matmul per 512-col chunk: d2 = x2 + y2 - 2 x.y).

Each 128-row x 2048-col d2 tile is cast PSUM->SBUF bf16 by the scalar
engine (or the vector engine for some tiles, balancing the two), then
either:
  - 'F' fold units: DVE min-accumulates the tile into a column-min
    accumulator and folds rows to [128,1024] width-2 pair-mins (host
    recomputes the winning pair's 2 columns exactly), or
  - '0' ship units: the raw bf16 tile goes to DRAM and the host does
    that tile's row- and column-reductions itself (the DMA queues have
    slack; GPSIMD can't help - neuronxcc rejects two-tensor Pool ops).
"""

import os
import sys

for _p in ("/opt/trn_rl_repo", "/root/.axon_site/_ro/trn_rl_repo"):
    if os.path.isdir(_p) and _p not in sys.path:
        sys.path.append(_p)

import numpy as np

# ---------------- problem constants (hardcoded) ----------------
B = 4
NSAMP = 4096          # sampled points per mesh (both pred and gt)
ROWS_PER_CORE = 2048  # pred rows per core (half a mesh)
T_TILES = 16          # row tiles of 128
MCOLS = 4096          # gt points per mesh
HALF = 2048           # column half processed per tile
N_UNITS = 32          # (2 col-halves) x (16 row-tiles)
N_CORES = 8

CHAMFER_W = 1.0
NORM_W = 0.1
EDGE_W = 0.5
EPS = 1e-12

# per-unit schedule: (feed, kind)
#  feed 'A': scalar engine casts PSUM f32 -> SBUF bf16
#  feed 'V': vector engine casts (tensor_scalar_max, clamps at 0)
#  kind 'F': fold unit - DVE colmin into the accumulator chain + one DVE
#            fold to [128,1024] pair-mins; host recomputes the 2 candidates
#  kind '0': ship unit - the raw bf16 tile goes to DRAM; the host does both
#            its row-side and column-side reductions
# (GPSIMD takes no part: neuronxcc rejects every two-tensor op on Pool.)
PLAN_HALF = [
    ("A", "F"),   # t0 seeds the accumulator (feed writes acc directly)
    ("A", "0"), ("V", "0"), ("A", "F"), ("A", "0"), ("V", "0"),
    ("A", "F"), ("A", "0"), ("V", "0"), ("A", "F"), ("A", "0"),
    ("A", "F"), ("V", "0"), ("A", "F"), ("V", "0"), ("A", "0"),
]
PLAN = PLAN_HALF + PLAN_HALF

# ---------------- bass program (built once) ----------------
_COMPILED = {}


def build_bass():
    import concourse.bacc as bacc
    import concourse.mybir as mybir
    import concourse.tile as tile

    f32 = mybir.dt.float32
    f32r = mybir.dt.float32r
    bf16 = mybir.dt.bfloat16
    amin = mybir.AluOpType.min

    nc = bacc.Bacc("TRN2", target_bir_lowering=False, debug=False)

    lhsT_d = nc.dram_tensor("lhsT", [5, ROWS_PER_CORE], f32r, kind="ExternalInput")
    rhs_d = nc.dram_tensor("rhs", [5, MCOLS], f32r, kind="ExternalInput")
    cm_d = nc.dram_tensor("cm", [N_UNITS, 128, HALF], bf16, kind="ExternalOutput")
    accD_d = nc.dram_tensor("accD", [128, MCOLS], bf16, kind="ExternalOutput")

    with tile.TileContext(nc) as tc:
        with (
            tc.tile_pool(name="ops", bufs=1) as ops_pool,
            tc.tile_pool(name="scopy", bufs=16) as s_pool,
            tc.tile_pool(name="cms", bufs=8) as cm_pool,
            tc.tile_pool(name="psum", bufs=4, space="PSUM") as psum_pool,
        ):
            lhsT_sb = ops_pool.tile([5, ROWS_PER_CORE], f32r)
            rhs_sb = ops_pool.tile([5, MCOLS], f32r)
            # input loads on the SP queue, head chunks first so the first
            # matmuls can start while the rest streams in (HWDGE generates
            # descriptors at ~625ns/DMA, so the order here is the head)
            nc.sync.dma_start(lhsT_sb[:, 0:128], lhsT_d[:, 0:128])
            nc.sync.dma_start(rhs_sb[:, 0:512], rhs_d[:, 0:512])
            nc.sync.dma_start(rhs_sb[:, 512:1024], rhs_d[:, 512:1024])
            nc.sync.dma_start(rhs_sb[:, 1024:2048], rhs_d[:, 1024:2048])
            nc.sync.dma_start(lhsT_sb[:, 128:2048], lhsT_d[:, 128:2048])
            nc.sync.dma_start(rhs_sb[:, 2048:4096], rhs_d[:, 2048:4096])

            accD = ops_pool.tile([128, MCOLS], bf16)

            # last fold unit per half -> ship that acc half right after it
            last_fold = {}
            for u, (feed, kind) in enumerate(PLAN):
                if kind == "F":
                    last_fold[u // T_TILES] = u

            def emit_reduction(u, s, do_colmin):
                """colmin + rowmin + stores for unit u (s = its bf16 tile)."""
                half = u // T_TILES
                feed, kind = PLAN[u]
                accD_h = accD[:, half * HALF:(half + 1) * HALF]
                if kind == "F":
                    if do_colmin:
                        nc.vector.tensor_tensor(accD_h, accD_h, s, op=amin)
                    cmq = cm_pool.tile([128, 1024], bf16, tag="cmq")
                    nc.vector.tensor_tensor(
                        cmq[:], s[:, 0:1024], s[:, 1024:2048], op=amin)
                    nc.sync.dma_start(cm_d[u, :, 0:1024], cmq[:])
                else:
                    nc.sync.dma_start(cm_d[u], s)
                if last_fold.get(half) == u:
                    nc.sync.dma_start(
                        accD_d[:, half * HALF:(half + 1) * HALF], accD_h)

            # software-pipelined emission: feeds go out immediately (so PSUM
            # recycles at feed pace and PE never starves); the reduction work
            # for a unit trails LAG units behind in each engine's stream
            LAG = 2
            seeded = {}
            pending = []
            for u in range(N_UNITS):
                half, t = divmod(u, T_TILES)
                feed, kind = PLAN[u]
                accD_h = accD[:, half * HALF:(half + 1) * HALF]

                # two [128,1024] psum tiles per unit (4 rotating slots in
                # the pool): each half is drained right after its 2 matmuls,
                # so PE is never gated on a whole-tile drain - the 2-slot
                # ping-pong was the pipeline's pacing item
                ps_parts = []
                for p in range(2):
                    psq = psum_pool.tile([128, 1024], f32, tag="psq")
                    ps_parts.append(psq)
                    for j in range(2):
                        c0 = half * HALF + p * 1024 + j * 512
                        nc.tensor.matmul(
                            psq[:, j * 512:(j + 1) * 512],
                            lhsT_sb[:, t * 128:(t + 1) * 128],
                            rhs_sb[:, c0:c0 + 512],
                            start=True,
                            stop=True,
                        )

                if kind == "F" and not seeded.get(half):
                    # chain seed: the feed writes the accumulator itself;
                    # this unit's fold later reads acc_h directly
                    seeded[half] = True
                    s = accD_h
                    do_colmin = False
                else:
                    s_tile = s_pool.tile([128, HALF], bf16, tag="scp")
                    s = s_tile[:]
                    do_colmin = True
                for p in range(2):
                    dst = s[:, p * 1024:(p + 1) * 1024]
                    use_act = feed == "A" or (feed == "M" and p == 0)
                    if use_act:
                        nc.scalar.copy(dst, ps_parts[p][:])
                    else:
                        nc.vector.tensor_scalar_max(dst, ps_parts[p][:], 0.0)

                pending.append((u, s, do_colmin))
                if len(pending) > LAG:
                    emit_reduction(*pending.pop(0))
            while pending:
                emit_reduction(*pending.pop(0))

    nc.compile()
    return nc


def _get_nc():
    if "nc" not in _COMPILED:
        _COMPILED["nc"] = build_bass()
    return _COMPILED["nc"]


# ---------------- host-side sampling (exact replica of reference) ----------------

def _sample_meshes(predicted_vertices, predicted_faces, gt_vertices, gt_faces):
    import jax
    import jax.numpy as jnp

    cpu = jax.devices("cpu")[0]

    def face_geometry(vertices, faces):
        v0 = vertices[:, faces[:, 0]]
        v1 = vertices[:, faces[:, 1]]
        v2 = vertices[:, faces[:, 2]]
        cross = jnp.cross(v1 - v0, v2 - v0)
        area2 = jnp.linalg.norm(cross, axis=-1)
        normals = cross / (area2[..., None] + EPS)
        return v0, v1, v2, 0.5 * area2, normals

    def sample_points(vertices, faces, n_samples, key):
        Bb = vertices.shape[0]
        v0, v1, v2, area, normals = face_geometry(vertices, faces)
        k_face, k_u, k_v = jax.random.split(key, 3)
        logits = jnp.log(area + EPS)
        face_idx = jax.random.categorical(
            k_face, logits[:, None, :], axis=-1, shape=(Bb, n_samples)
        )
        gather = lambda a: jnp.take_along_axis(a, face_idx[..., None], axis=1)
        p0, p1, p2 = gather(v0), gather(v1), gather(v2)
        u = jax.random.uniform(k_u, (Bb, n_samples, 1))
        v = jax.random.uniform(k_v, (Bb, n_samples, 1))
        r1 = jnp.sqrt(u)
        points = (1.0 - r1) * p0 + r1 * (1.0 - v) * p1 + r1 * v * p2
        point_normals = gather(normals)
        return points, point_normals

    def sample_all(pv, pf, gv, gf):
        key = jax.random.key(42)
        kp, kg = jax.random.split(key)
        pred_pts, pred_nrm = sample_points(pv, pf, NSAMP, kp)
        gt_pts, gt_nrm = sample_points(gv, gf, NSAMP, kg)
        return pred_pts, pred_nrm, gt_pts, gt_nrm

    fn = _COMPILED.get("sample_jit")
    if fn is None:
        fn = jax.jit(sample_all, backend="cpu")
        _COMPILED["sample_jit"] = fn

    with jax.default_device(cpu):
        out = fn(
            jnp.asarray(predicted_vertices), jnp.asarray(predicted_faces),
            jnp.asarray(gt_vertices), jnp.asarray(gt_faces),
        )
        out = tuple(np.asarray(a) for a in out)
    return out


# ---------------- main entry ----------------

def kernel(predicted_vertices, predicted_faces, gt_vertices, gt_faces):
    from concourse.bass_utils import run_bass_kernel_spmd

    predicted_vertices = np.asarray(predicted_vertices, dtype=np.float32)
    gt_vertices = np.asarray(gt_vertices, dtype=np.float32)

    pred_pts, pred_nrm, gt_pts, gt_nrm = _sample_meshes(
        predicted_vertices, predicted_faces, gt_vertices, gt_faces
    )

    # per-core operands: core c -> mesh b = c//2, row half rh = c%2
    x2_all = np.sum(pred_pts * pred_pts, axis=-1)  # [B, 4096]
    y2_all = np.sum(gt_pts * gt_pts, axis=-1)      # [B, 4096]

    in_maps = []
    for c in range(N_CORES):
        b, rh = divmod(c, 2)
        x = pred_pts[b, rh * ROWS_PER_CORE:(rh + 1) * ROWS_PER_CORE]
        y = gt_pts[b]
        x2 = x2_all[b, rh * ROWS_PER_CORE:(rh + 1) * ROWS_PER_CORE]
        y2 = y2_all[b]
        lhsT = np.empty((5, ROWS_PER_CORE), np.float32)
        lhsT[0:3] = -2.0 * x.T
        lhsT[3] = x2
        lhsT[4] = 1.0
        rhs = np.empty((5, MCOLS), np.float32)
        rhs[0:3] = y.T
        rhs[3] = 1.0
        rhs[4] = y2
        in_maps.append({"lhsT": lhsT, "rhs": rhs})

    nc = _get_nc()
    res = run_bass_kernel_spmd(nc, in_maps, list(range(N_CORES))).results

    # ---------------- host postprocessing ----------------
    min_x2y = np.empty((B, NSAMP), np.float32)
    idx_p2g = np.empty((B, NSAMP), np.int64)
    min_y2x = np.empty((B, MCOLS), np.float32)

    rows_l = np.arange(ROWS_PER_CORE)
    # per unit: raw-shipped tile ('0') vs on-device width-2 pair-mins ('F')
    shipped = np.array([PLAN[u][1] == "0" for u in range(N_UNITS)])
    for b in range(B):
        col_partials = []
        for rh in range(2):
            r = res[2 * b + rh]
            cm = np.asarray(r["cm"], np.float32)           # [32, 128, 2048]
            # device column partial (fold units' rows) + shipped tiles
            aD = np.asarray(r["accD"], np.float32)         # [128, 4096]
            colmin = aD.min(axis=0)                        # [4096]
            # reduce each unit to (best value, candidate col pair) per row
            bv = np.empty((2, T_TILES, 128), np.float32)
            bc = np.empty((2, T_TILES, 128, 2), np.int64)
            for u in range(N_UNITS):
                h, t = divmod(u, T_TILES)
                base = h * HALF
                if shipped[u]:
                    j = np.argmin(cm[u], axis=1)           # [128]
                    bv[h, t] = cm[u][np.arange(128), j]
                    bc[h, t, :, 0] = base + j
                    bc[h, t, :, 1] = base + j
                    np.minimum(colmin[base:base + HALF], cm[u].min(axis=0),
                               out=colmin[base:base + HALF])
                else:
                    M = cm[u, :, 0:1024]
                    j = np.argmin(M, axis=1)
                    bv[h, t] = M[np.arange(128), j]
                    bc[h, t, :, 0] = base + j
                    bc[h, t, :, 1] = base + j + 1024
            col_partials.append(colmin)
            # pick the winning half per row, recompute its <=2 candidates
            hwin = np.argmin(bv, axis=0).reshape(ROWS_PER_CORE)   # [2048]
            cand = bc.transpose(1, 2, 0, 3).reshape(ROWS_PER_CORE, 2, 2)
            cand = cand[rows_l, hwin]                      # [2048, 2]

            xb = pred_pts[b, rh * ROWS_PER_CORE:(rh + 1) * ROWS_PER_CORE]
            ycand = gt_pts[b][cand]                        # [2048, 2, 3]
            d2c = (
                x2_all[b, rh * ROWS_PER_CORE:(rh + 1) * ROWS_PER_CORE][:, None]
                + y2_all[b][cand]
                - 2.0 * np.einsum("nd,nkd->nk", xb, ycand, dtype=np.float32)
            ).astype(np.float32)
            d2c = np.maximum(d2c, 0.0)
            within = np.argmin(d2c, axis=1)
            sl = slice(rh * ROWS_PER_CORE, (rh + 1) * ROWS_PER_CORE)
            min_x2y[b, sl] = d2c[rows_l, within]
            idx_p2g[b, sl] = cand[rows_l, within]

        acc_b = np.minimum(col_partials[0], col_partials[1])
        min_y2x[b] = np.maximum(acc_b, 0.0)

    chamfer = np.float32(np.mean(min_x2y)) + np.float32(np.mean(min_y2x))

    # normal consistency
    matched = np.take_along_axis(gt_nrm, idx_p2g[..., None], axis=1)
    cos = np.abs(np.sum(pred_nrm * matched, axis=-1))
    normal_loss = np.float32(np.mean(1.0 - cos))

    # edge loss (exact, on host)
    pf = np.asarray(predicted_faces).astype(np.int64)
    v0 = predicted_vertices[:, pf[:, 0]]
    v1 = predicted_vertices[:, pf[:, 1]]
    v2 = predicted_vertices[:, pf[:, 2]]
    e = np.concatenate([v1 - v0, v2 - v1, v0 - v2], axis=1)
    edge_loss = np.float32(np.mean(np.sum(e * e, axis=-1)))

    total = (
        np.float32(CHAMFER_W) * chamfer
        + np.float32(NORM_W) * normal_loss
        + np.float32(EDGE_W) * edge_loss
    )
    return np.asarray(total, dtype=np.float32)


# revision 53
# speedup vs baseline: 1.1597x; 1.0141x over previous
"""Mesh chamfer/normal/edge loss on 8 Trainium2 NeuronCores.

Sharding: data-parallel over the 4 meshes x 2 row-halves -> 8 cores.
Each core computes its 2048x4096 squared-distance matrix on-device
(single K=5 matmul per 512-col chunk: d2 = x2 + y2 - 2 x.y).

Each 128-row x 2048-col d2 tile is cast PSUM->SBUF bf16 by the scalar
engine (or the vector engine for some tiles, balancing the two), then
either:
  - 'F' fold units: DVE min-accumulates the tile into a column-min
    accumulator and folds rows to [128,1024] width-2 pair-mins (host
    recomputes the winning pair's 2 columns exactly), or
  - '0' ship units: the raw bf16 tile goes to DRAM and the host does
    that tile's row- and column-reductions itself (the DMA queues have
    slack; GPSIMD can't help - neuronxcc rejects two-tensor Pool ops).
"""

import os
import sys

for _p in ("/opt/trn_rl_repo", "/root/.axon_site/_ro/trn_rl_repo"):
    if os.path.isdir(_p) and _p not in sys.path:
        sys.path.append(_p)

import numpy as np

# ---------------- problem constants (hardcoded) ----------------
B = 4
NSAMP = 4096          # sampled points per mesh (both pred and gt)
ROWS_PER_CORE = 2048  # pred rows per core (half a mesh)
T_TILES = 16          # row tiles of 128
MCOLS = 4096          # gt points per mesh
HALF = 2048           # column half processed per tile
N_UNITS = 32          # (2 col-halves) x (16 row-tiles)
N_CORES = 8

CHAMFER_W = 1.0
NORM_W = 0.1
EDGE_W = 0.5
EPS = 1e-12

# per-unit schedule: (feed, kind)
#  feed 'A': scalar engine casts PSUM f32 -> SBUF bf16
#  feed 'V': vector engine casts (tensor_scalar_max, clamps at 0)
#  kind 'F': fold unit - DVE colmin into the accumulator chain + one DVE
#            fold to [128,1024] pair-mins; host recomputes the 2 candidates
#  kind '0': ship unit - the raw bf16 tile goes to DRAM; the host does both
#            its row-side and column-side reductions
# (GPSIMD takes no part: neuronxcc rejects every two-tensor op on Pool.)
PLAN_HALF = [
    ("A", "F"),   # t0 seeds the accumulator (feed writes acc directly)
    ("A", "0"), ("V", "0"), ("A", "F"), ("A", "0"), ("V", "0"),
    ("A", "F"), ("A", "0"), ("V", "0"), ("A", "F"), ("A", "0"),
    ("A", "F"), ("V", "0"), ("A", "F"), ("V", "0"), ("A", "0"),
]
PLAN = PLAN_HALF + PLAN_HALF

# ---------------- bass program (built once) ----------------
_COMPILED = {}


def build_bass():
    import concourse.bacc as bacc
    import concourse.mybir as mybir
    import concourse.tile as tile

    f32 = mybir.dt.float32
    f32r = mybir.dt.float32r
    bf16 = mybir.dt.bfloat16
    amin = mybir.AluOpType.min

    nc = bacc.Bacc("TRN2", target_bir_lowering=False, debug=False)

    lhsT_d = nc.dram_tensor("lhsT", [5, ROWS_PER_CORE], f32r, kind="ExternalInput")
    rhs_d = nc.dram_tensor("rhs", [5, MCOLS], f32r, kind="ExternalInput")
    cm_d = nc.dram_tensor("cm", [N_UNITS, 128, HALF], bf16, kind="ExternalOutput")
    accD_d = nc.dram_tensor("accD", [128, MCOLS], bf16, kind="ExternalOutput")

    with tile.TileContext(nc) as tc:
        with (
            tc.tile_pool(name="ops", bufs=1) as ops_pool,
            tc.tile_pool(name="scopy", bufs=16) as s_pool,
            tc.tile_pool(name="cms", bufs=8) as cm_pool,
            tc.tile_pool(name="psum", bufs=4, space="PSUM") as psum_pool,
        ):
            lhsT_sb = ops_pool.tile([5, ROWS_PER_CORE], f32r)
            rhs_sb = ops_pool.tile([5, MCOLS], f32r)
            # input loads on the SP queue, head chunks first so the first
            # matmuls can start while the rest streams in (HWDGE generates
            # descriptors at ~625ns/DMA, so the order here is the head)
            nc.sync.dma_start(lhsT_sb[:, 0:128], lhsT_d[:, 0:128])
            nc.sync.dma_start(rhs_sb[:, 0:512], rhs_d[:, 0:512])
            nc.sync.dma_start(rhs_sb[:, 512:1024], rhs_d[:, 512:1024])
            nc.sync.dma_start(rhs_sb[:, 1024:2048], rhs_d[:, 1024:2048])
            nc.sync.dma_start(lhsT_sb[:, 128:2048], lhsT_d[:, 128:2048])
            nc.sync.dma_start(rhs_sb[:, 2048:4096], rhs_d[:, 2048:4096])

            accD = ops_pool.tile([128, MCOLS], bf16)

            # last fold unit per half -> ship that acc half right after it
            last_fold = {}
            for u, (feed, kind) in enumerate(PLAN):
                if kind == "F":
                    last_fold[u // T_TILES] = u

            def emit_reduction(u, s, do_colmin):
                """colmin + rowmin + stores for unit u (s = its bf16 tile)."""
                half = u // T_TILES
                feed, kind = PLAN[u]
                accD_h = accD[:, half * HALF:(half + 1) * HALF]
                if kind == "F":
                    if do_colmin:
                        nc.vector.tensor_tensor(accD_h, accD_h, s, op=amin)
                    cmq = cm_pool.tile([128, 1024], bf16, tag="cmq")
                    nc.vector.tensor_tensor(
                        cmq[:], s[:, 0:1024], s[:, 1024:2048], op=amin)
                    nc.sync.dma_start(cm_d[u, :, 0:1024], cmq[:])
                else:
                    # two half-stores: each depends only on its own feed
                    # part, so the store stream starts earlier
                    nc.sync.dma_start(cm_d[u, :, 0:1024], s[:, 0:1024])
                    nc.sync.dma_start(cm_d[u, :, 1024:2048], s[:, 1024:2048])
                if last_fold.get(half) == u:
                    nc.sync.dma_start(
                        accD_d[:, half * HALF:(half + 1) * HALF], accD_h)

            # software-pipelined emission: feeds go out immediately (so PSUM
            # recycles at feed pace and PE never starves); the reduction work
            # for a unit trails LAG units behind in each engine's stream
            LAG = 2
            seeded = {}
            pending = []
            for u in range(N_UNITS):
                half, t = divmod(u, T_TILES)
                feed, kind = PLAN[u]
                accD_h = accD[:, half * HALF:(half + 1) * HALF]

                # two [128,1024] psum tiles per unit (4 rotating slots in
                # the pool): each half is drained right after its 2 matmuls,
                # so PE is never gated on a whole-tile drain - the 2-slot
                # ping-pong was the pipeline's pacing item
                ps_parts = []
                for p in range(2):
                    psq = psum_pool.tile([128, 1024], f32, tag="psq")
                    ps_parts.append(psq)
                    for j in range(2):
                        c0 = half * HALF + p * 1024 + j * 512
                        nc.tensor.matmul(
                            psq[:, j * 512:(j + 1) * 512],
                            lhsT_sb[:, t * 128:(t + 1) * 128],
                            rhs_sb[:, c0:c0 + 512],
                            start=True,
                            stop=True,
                        )

                if kind == "F" and not seeded.get(half):
                    # chain seed: the feed writes the accumulator itself;
                    # this unit's fold later reads acc_h directly
                    seeded[half] = True
                    s = accD_h
                    do_colmin = False
                else:
                    s_tile = s_pool.tile([128, HALF], bf16, tag="scp")
                    s = s_tile[:]
                    do_colmin = True
                for p in range(2):
                    dst = s[:, p * 1024:(p + 1) * 1024]
                    use_act = feed == "A" or (feed == "M" and p == 0)
                    if use_act:
                        nc.scalar.copy(dst, ps_parts[p][:])
                    else:
                        nc.vector.tensor_scalar_max(dst, ps_parts[p][:], 0.0)

                pending.append((u, s, do_colmin))
                if len(pending) > LAG:
                    emit_reduction(*pending.pop(0))
            while pending:
                emit_reduction(*pending.pop(0))

    nc.compile()
    return nc


def _get_nc():
    if "nc" not in _COMPILED:
        _COMPILED["nc"] = build_bass()
    return _COMPILED["nc"]


# ---------------- host-side sampling (exact replica of reference) ----------------

def _sample_meshes(predicted_vertices, predicted_faces, gt_vertices, gt_faces):
    import jax
    import jax.numpy as jnp

    cpu = jax.devices("cpu")[0]

    def face_geometry(vertices, faces):
        v0 = vertices[:, faces[:, 0]]
        v1 = vertices[:, faces[:, 1]]
        v2 = vertices[:, faces[:, 2]]
        cross = jnp.cross(v1 - v0, v2 - v0)
        area2 = jnp.linalg.norm(cross, axis=-1)
        normals = cross / (area2[..., None] + EPS)
        return v0, v1, v2, 0.5 * area2, normals

    def sample_points(vertices, faces, n_samples, key):
        Bb = vertices.shape[0]
        v0, v1, v2, area, normals = face_geometry(vertices, faces)
        k_face, k_u, k_v = jax.random.split(key, 3)
        logits = jnp.log(area + EPS)
        face_idx = jax.random.categorical(
            k_face, logits[:, None, :], axis=-1, shape=(Bb, n_samples)
        )
        gather = lambda a: jnp.take_along_axis(a, face_idx[..., None], axis=1)
        p0, p1, p2 = gather(v0), gather(v1), gather(v2)
        u = jax.random.uniform(k_u, (Bb, n_samples, 1))
        v = jax.random.uniform(k_v, (Bb, n_samples, 1))
        r1 = jnp.sqrt(u)
        points = (1.0 - r1) * p0 + r1 * (1.0 - v) * p1 + r1 * v * p2
        point_normals = gather(normals)
        return points, point_normals

    def sample_all(pv, pf, gv, gf):
        key = jax.random.key(42)
        kp, kg = jax.random.split(key)
        pred_pts, pred_nrm = sample_points(pv, pf, NSAMP, kp)
        gt_pts, gt_nrm = sample_points(gv, gf, NSAMP, kg)
        return pred_pts, pred_nrm, gt_pts, gt_nrm

    fn = _COMPILED.get("sample_jit")
    if fn is None:
        fn = jax.jit(sample_all, backend="cpu")
        _COMPILED["sample_jit"] = fn

    with jax.default_device(cpu):
        out = fn(
            jnp.asarray(predicted_vertices), jnp.asarray(predicted_faces),
            jnp.asarray(gt_vertices), jnp.asarray(gt_faces),
        )
        out = tuple(np.asarray(a) for a in out)
    return out


# ---------------- main entry ----------------

def kernel(predicted_vertices, predicted_faces, gt_vertices, gt_faces):
    from concourse.bass_utils import run_bass_kernel_spmd

    predicted_vertices = np.asarray(predicted_vertices, dtype=np.float32)
    gt_vertices = np.asarray(gt_vertices, dtype=np.float32)

    pred_pts, pred_nrm, gt_pts, gt_nrm = _sample_meshes(
        predicted_vertices, predicted_faces, gt_vertices, gt_faces
    )

    # per-core operands: core c -> mesh b = c//2, row half rh = c%2
    x2_all = np.sum(pred_pts * pred_pts, axis=-1)  # [B, 4096]
    y2_all = np.sum(gt_pts * gt_pts, axis=-1)      # [B, 4096]

    in_maps = []
    for c in range(N_CORES):
        b, rh = divmod(c, 2)
        x = pred_pts[b, rh * ROWS_PER_CORE:(rh + 1) * ROWS_PER_CORE]
        y = gt_pts[b]
        x2 = x2_all[b, rh * ROWS_PER_CORE:(rh + 1) * ROWS_PER_CORE]
        y2 = y2_all[b]
        lhsT = np.empty((5, ROWS_PER_CORE), np.float32)
        lhsT[0:3] = -2.0 * x.T
        lhsT[3] = x2
        lhsT[4] = 1.0
        rhs = np.empty((5, MCOLS), np.float32)
        rhs[0:3] = y.T
        rhs[3] = 1.0
        rhs[4] = y2
        in_maps.append({"lhsT": lhsT, "rhs": rhs})

    nc = _get_nc()
    res = run_bass_kernel_spmd(nc, in_maps, list(range(N_CORES))).results

    # ---------------- host postprocessing ----------------
    min_x2y = np.empty((B, NSAMP), np.float32)
    idx_p2g = np.empty((B, NSAMP), np.int64)
    min_y2x = np.empty((B, MCOLS), np.float32)

    rows_l = np.arange(ROWS_PER_CORE)
    # per unit: raw-shipped tile ('0') vs on-device width-2 pair-mins ('F')
    shipped = np.array([PLAN[u][1] == "0" for u in range(N_UNITS)])
    for b in range(B):
        col_partials = []
        for rh in range(2):
            r = res[2 * b + rh]
            cm = np.asarray(r["cm"], np.float32)           # [32, 128, 2048]
            # device column partial (fold units' rows) + shipped tiles
            aD = np.asarray(r["accD"], np.float32)         # [128, 4096]
            colmin = aD.min(axis=0)                        # [4096]
            # reduce each unit to (best value, candidate col pair) per row
            bv = np.empty((2, T_TILES, 128), np.float32)
            bc = np.empty((2, T_TILES, 128, 2), np.int64)
            for u in range(N_UNITS):
                h, t = divmod(u, T_TILES)
                base = h * HALF
                if shipped[u]:
                    j = np.argmin(cm[u], axis=1)           # [128]
                    bv[h, t] = cm[u][np.arange(128), j]
                    bc[h, t, :, 0] = base + j
                    bc[h, t, :, 1] = base + j
                    np.minimum(colmin[base:base + HALF], cm[u].min(axis=0),
                               out=colmin[base:base + HALF])
                else:
                    M = cm[u, :, 0:1024]
                    j = np.argmin(M, axis=1)
                    bv[h, t] = M[np.arange(128), j]
                    bc[h, t, :, 0] = base + j
                    bc[h, t, :, 1] = base + j + 1024
            col_partials.append(colmin)
            # pick the winning half per row, recompute its <=2 candidates
            hwin = np.argmin(bv, axis=0).reshape(ROWS_PER_CORE)   # [2048]
            cand = bc.transpose(1, 2, 0, 3).reshape(ROWS_PER_CORE, 2, 2)
            cand = cand[rows_l, hwin]                      # [2048, 2]

            xb = pred_pts[b, rh * ROWS_PER_CORE:(rh + 1) * ROWS_PER_CORE]
            ycand = gt_pts[b][cand]                        # [2048, 2, 3]
            d2c = (
                x2_all[b, rh * ROWS_PER_CORE:(rh + 1) * ROWS_PER_CORE][:, None]
                + y2_all[b][cand]
                - 2.0 * np.einsum("nd,nkd->nk", xb, ycand, dtype=np.float32)
            ).astype(np.float32)
            d2c = np.maximum(d2c, 0.0)
            within = np.argmin(d2c, axis=1)
            sl = slice(rh * ROWS_PER_CORE, (rh + 1) * ROWS_PER_CORE)
            min_x2y[b, sl] = d2c[rows_l, within]
            idx_p2g[b, sl] = cand[rows_l, within]

        acc_b = np.minimum(col_partials[0], col_partials[1])
        min_y2x[b] = np.maximum(acc_b, 0.0)

    chamfer = np.float32(np.mean(min_x2y)) + np.float32(np.mean(min_y2x))

    # normal consistency
    matched = np.take_along_axis(gt_nrm, idx_p2g[..., None], axis=1)
    cos = np.abs(np.sum(pred_nrm * matched, axis=-1))
    normal_loss = np.float32(np.mean(1.0 - cos))

    # edge loss (exact, on host)
    pf = np.asarray(predicted_faces).astype(np.int64)
    v0 = predicted_vertices[:, pf[:, 0]]
    v1 = predicted_vertices[:, pf[:, 1]]
    v2 = predicted_vertices[:, pf[:, 2]]
    e = np.concatenate([v1 - v0, v2 - v1, v0 - v2], axis=1)
    edge_loss = np.float32(np.mean(np.sum(e * e, axis=-1)))

    total = (
        np.float32(CHAMFER_W) * chamfer
        + np.float32(NORM_W) * normal_loss
        + np.float32(EDGE_W) * edge_loss
    )
    return np.asarray(total, dtype=np.float32)


# revision 58
# speedup vs baseline: 1.1721x; 1.0107x over previous
"""Mesh chamfer/normal/edge loss on 8 Trainium2 NeuronCores.

Sharding: data-parallel over the 4 meshes x 2 row-halves -> 8 cores.
Each core computes its 2048x4096 squared-distance matrix on-device
(single K=5 matmul per 512-col chunk: d2 = x2 + y2 - 2 x.y).

Each 128-row x 2048-col d2 tile is cast PSUM->SBUF bf16 by the scalar
engine (or the vector engine for some tiles, balancing the two), then
either:
  - 'F' fold units: DVE min-accumulates the tile into a column-min
    accumulator and folds rows to [128,1024] width-2 pair-mins (host
    recomputes the winning pair's 2 columns exactly), or
  - '0' ship units: the raw bf16 tile goes to DRAM and the host does
    that tile's row- and column-reductions itself (the DMA queues have
    slack; GPSIMD can't help - neuronxcc rejects two-tensor Pool ops).
"""

import os
import sys

for _p in ("/opt/trn_rl_repo", "/root/.axon_site/_ro/trn_rl_repo"):
    if os.path.isdir(_p) and _p not in sys.path:
        sys.path.append(_p)

import numpy as np

# ---------------- problem constants (hardcoded) ----------------
B = 4
NSAMP = 4096          # sampled points per mesh (both pred and gt)
ROWS_PER_CORE = 2048  # pred rows per core (half a mesh)
T_TILES = 16          # row tiles of 128
MCOLS = 4096          # gt points per mesh
HALF = 2048           # column half processed per tile
N_UNITS = 32          # (2 col-halves) x (16 row-tiles)
N_CORES = 8

CHAMFER_W = 1.0
NORM_W = 0.1
EDGE_W = 0.5
EPS = 1e-12

# per-unit schedule: (feed, kind)
#  feed 'A': scalar engine casts PSUM f32 -> SBUF bf16
#  feed 'V': vector engine casts (tensor_scalar_max, clamps at 0)
#  kind 'F': fold unit - DVE colmin into the accumulator chain + one DVE
#            fold to [128,1024] pair-mins; host recomputes the 2 candidates
#  kind '0': ship unit - the raw bf16 tile goes to DRAM; the host does both
#            its row-side and column-side reductions
# (GPSIMD takes no part: neuronxcc rejects every two-tensor op on Pool.)
PLAN_HALF = [
    ("A", "F"),   # t0 seeds the accumulator (feed writes acc directly)
    ("A", "0"), ("V", "0"), ("A", "F"), ("A", "0"), ("V", "0"),
    ("A", "F"), ("A", "0"), ("V", "0"), ("A", "F"), ("A", "0"),
    ("A", "F"), ("V", "0"), ("A", "F"), ("V", "0"), ("A", "0"),
]
# half 1 keeps the same mix but ends DVE-light: no V-feeds or fold units
# in the last three slots, so the vector engine's trailing reduction work
# doesn't extend the kernel past the scalar engine's last feed
PLAN_HALF1 = [
    ("A", "F"), ("A", "0"), ("V", "0"), ("A", "F"), ("V", "0"), ("A", "0"),
    ("A", "F"), ("V", "0"), ("A", "0"), ("A", "F"), ("V", "0"), ("A", "F"),
    ("V", "0"), ("A", "F"), ("A", "0"), ("A", "0"),
]
PLAN = PLAN_HALF + PLAN_HALF1

# ---------------- bass program (built once) ----------------
_COMPILED = {}


def build_bass():
    import concourse.bacc as bacc
    import concourse.mybir as mybir
    import concourse.tile as tile

    f32 = mybir.dt.float32
    f32r = mybir.dt.float32r
    bf16 = mybir.dt.bfloat16
    amin = mybir.AluOpType.min

    nc = bacc.Bacc("TRN2", target_bir_lowering=False, debug=False)

    lhsT_d = nc.dram_tensor("lhsT", [5, ROWS_PER_CORE], f32r, kind="ExternalInput")
    rhs_d = nc.dram_tensor("rhs", [5, MCOLS], f32r, kind="ExternalInput")
    cm_d = nc.dram_tensor("cm", [N_UNITS, 128, HALF], bf16, kind="ExternalOutput")
    accD_d = nc.dram_tensor("accD", [128, MCOLS], bf16, kind="ExternalOutput")

    with tile.TileContext(nc) as tc:
        with (
            tc.tile_pool(name="ops", bufs=1) as ops_pool,
            tc.tile_pool(name="scopy", bufs=16) as s_pool,
            tc.tile_pool(name="cms", bufs=8) as cm_pool,
            tc.tile_pool(name="psum", bufs=4, space="PSUM") as psum_pool,
        ):
            lhsT_sb = ops_pool.tile([5, ROWS_PER_CORE], f32r)
            rhs_sb = ops_pool.tile([5, MCOLS], f32r)
            # input loads on the SP queue, head chunks first so the first
            # matmuls can start while the rest streams in (HWDGE generates
            # descriptors at ~625ns/DMA, so the order here is the head)
            nc.sync.dma_start(lhsT_sb[:, 0:128], lhsT_d[:, 0:128])
            nc.sync.dma_start(rhs_sb[:, 0:512], rhs_d[:, 0:512])
            nc.sync.dma_start(rhs_sb[:, 512:1024], rhs_d[:, 512:1024])
            nc.sync.dma_start(rhs_sb[:, 1024:2048], rhs_d[:, 1024:2048])
            nc.sync.dma_start(lhsT_sb[:, 128:2048], lhsT_d[:, 128:2048])
            nc.sync.dma_start(rhs_sb[:, 2048:4096], rhs_d[:, 2048:4096])

            accD = ops_pool.tile([128, MCOLS], bf16)

            # last fold unit per half -> ship that acc half right after it
            last_fold = {}
            for u, (feed, kind) in enumerate(PLAN):
                if kind == "F":
                    last_fold[u // T_TILES] = u

            def emit_reduction(u, s, do_colmin):
                """colmin + rowmin + stores for unit u (s = its bf16 tile)."""
                half = u // T_TILES
                feed, kind = PLAN[u]
                accD_h = accD[:, half * HALF:(half + 1) * HALF]
                if kind == "F":
                    if do_colmin:
                        nc.vector.tensor_tensor(accD_h, accD_h, s, op=amin)
                    cmq = cm_pool.tile([128, 1024], bf16, tag="cmq")
                    nc.vector.tensor_tensor(
                        cmq[:], s[:, 0:1024], s[:, 1024:2048], op=amin)
                    nc.sync.dma_start(cm_d[u, :, 0:1024], cmq[:])
                else:
                    # two half-stores: each depends only on its own feed
                    # part, so the store stream starts earlier
                    nc.sync.dma_start(cm_d[u, :, 0:1024], s[:, 0:1024])
                    nc.sync.dma_start(cm_d[u, :, 1024:2048], s[:, 1024:2048])
                if last_fold.get(half) == u:
                    nc.sync.dma_start(
                        accD_d[:, half * HALF:(half + 1) * HALF], accD_h)

            # software-pipelined emission: feeds go out immediately (so PSUM
            # recycles at feed pace and PE never starves); the reduction work
            # for a unit trails LAG units behind in each engine's stream
            LAG = 2
            seeded = {}
            pending = []
            for u in range(N_UNITS):
                half, t = divmod(u, T_TILES)
                feed, kind = PLAN[u]
                accD_h = accD[:, half * HALF:(half + 1) * HALF]

                # two [128,1024] psum tiles per unit (4 rotating slots in
                # the pool): each half is drained right after its 2 matmuls,
                # so PE is never gated on a whole-tile drain - the 2-slot
                # ping-pong was the pipeline's pacing item
                ps_parts = []
                for p in range(2):
                    psq = psum_pool.tile([128, 1024], f32, tag="psq")
                    ps_parts.append(psq)
                    for j in range(2):
                        c0 = half * HALF + p * 1024 + j * 512
                        nc.tensor.matmul(
                            psq[:, j * 512:(j + 1) * 512],
                            lhsT_sb[:, t * 128:(t + 1) * 128],
                            rhs_sb[:, c0:c0 + 512],
                            start=True,
                            stop=True,
                        )

                if kind == "F" and not seeded.get(half):
                    # chain seed: the feed writes the accumulator itself;
                    # this unit's fold later reads acc_h directly
                    seeded[half] = True
                    s = accD_h
                    do_colmin = False
                else:
                    s_tile = s_pool.tile([128, HALF], bf16, tag="scp")
                    s = s_tile[:]
                    do_colmin = True
                for p in range(2):
                    dst = s[:, p * 1024:(p + 1) * 1024]
                    use_act = feed == "A" or (feed == "M" and p == 0)
                    if use_act:
                        nc.scalar.copy(dst, ps_parts[p][:])
                    else:
                        nc.vector.tensor_scalar_max(dst, ps_parts[p][:], 0.0)

                pending.append((u, s, do_colmin))
                if len(pending) > LAG:
                    emit_reduction(*pending.pop(0))
            while pending:
                emit_reduction(*pending.pop(0))

    nc.compile()
    return nc


def _get_nc():
    if "nc" not in _COMPILED:
        _COMPILED["nc"] = build_bass()
    return _COMPILED["nc"]


# ---------------- host-side sampling (exact replica of reference) ----------------

def _sample_meshes(predicted_vertices, predicted_faces, gt_vertices, gt_faces):
    import jax
    import jax.numpy as jnp

    cpu = jax.devices("cpu")[0]

    def face_geometry(vertices, faces):
        v0 = vertices[:, faces[:, 0]]
        v1 = vertices[:, faces[:, 1]]
        v2 = vertices[:, faces[:, 2]]
        cross = jnp.cross(v1 - v0, v2 - v0)
        area2 = jnp.linalg.norm(cross, axis=-1)
        normals = cross / (area2[..., None] + EPS)
        return v0, v1, v2, 0.5 * area2, normals

    def sample_points(vertices, faces, n_samples, key):
        Bb = vertices.shape[0]
        v0, v1, v2, area, normals = face_geometry(vertices, faces)
        k_face, k_u, k_v = jax.random.split(key, 3)
        logits = jnp.log(area + EPS)
        face_idx = jax.random.categorical(
            k_face, logits[:, None, :], axis=-1, shape=(Bb, n_samples)
        )
        gather = lambda a: jnp.take_along_axis(a, face_idx[..., None], axis=1)
        p0, p1, p2 = gather(v0), gather(v1), gather(v2)
        u = jax.random.uniform(k_u, (Bb, n_samples, 1))
        v = jax.random.uniform(k_v, (Bb, n_samples, 1))
        r1 = jnp.sqrt(u)
        points = (1.0 - r1) * p0 + r1 * (1.0 - v) * p1 + r1 * v * p2
        point_normals = gather(normals)
        return points, point_normals

    def sample_all(pv, pf, gv, gf):
        key = jax.random.key(42)
        kp, kg = jax.random.split(key)
        pred_pts, pred_nrm = sample_points(pv, pf, NSAMP, kp)
        gt_pts, gt_nrm = sample_points(gv, gf, NSAMP, kg)
        return pred_pts, pred_nrm, gt_pts, gt_nrm

    fn = _COMPILED.get("sample_jit")
    if fn is None:
        fn = jax.jit(sample_all, backend="cpu")
        _COMPILED["sample_jit"] = fn

    with jax.default_device(cpu):
        out = fn(
            jnp.asarray(predicted_vertices), jnp.asarray(predicted_faces),
            jnp.asarray(gt_vertices), jnp.asarray(gt_faces),
        )
        out = tuple(np.asarray(a) for a in out)
    return out


# ---------------- main entry ----------------

def kernel(predicted_vertices, predicted_faces, gt_vertices, gt_faces):
    from concourse.bass_utils import run_bass_kernel_spmd

    predicted_vertices = np.asarray(predicted_vertices, dtype=np.float32)
    gt_vertices = np.asarray(gt_vertices, dtype=np.float32)

    pred_pts, pred_nrm, gt_pts, gt_nrm = _sample_meshes(
        predicted_vertices, predicted_faces, gt_vertices, gt_faces
    )

    # per-core operands: core c -> mesh b = c//2, row half rh = c%2
    x2_all = np.sum(pred_pts * pred_pts, axis=-1)  # [B, 4096]
    y2_all = np.sum(gt_pts * gt_pts, axis=-1)      # [B, 4096]

    in_maps = []
    for c in range(N_CORES):
        b, rh = divmod(c, 2)
        x = pred_pts[b, rh * ROWS_PER_CORE:(rh + 1) * ROWS_PER_CORE]
        y = gt_pts[b]
        x2 = x2_all[b, rh * ROWS_PER_CORE:(rh + 1) * ROWS_PER_CORE]
        y2 = y2_all[b]
        lhsT = np.empty((5, ROWS_PER_CORE), np.float32)
        lhsT[0:3] = -2.0 * x.T
        lhsT[3] = x2
        lhsT[4] = 1.0
        rhs = np.empty((5, MCOLS), np.float32)
        rhs[0:3] = y.T
        rhs[3] = 1.0
        rhs[4] = y2
        in_maps.append({"lhsT": lhsT, "rhs": rhs})

    nc = _get_nc()
    res = run_bass_kernel_spmd(nc, in_maps, list(range(N_CORES))).results

    # ---------------- host postprocessing ----------------
    min_x2y = np.empty((B, NSAMP), np.float32)
    idx_p2g = np.empty((B, NSAMP), np.int64)
    min_y2x = np.empty((B, MCOLS), np.float32)

    rows_l = np.arange(ROWS_PER_CORE)
    # per unit: raw-shipped tile ('0') vs on-device width-2 pair-mins ('F')
    shipped = np.array([PLAN[u][1] == "0" for u in range(N_UNITS)])
    for b in range(B):
        col_partials = []
        for rh in range(2):
            r = res[2 * b + rh]
            cm = np.asarray(r["cm"], np.float32)           # [32, 128, 2048]
            # device column partial (fold units' rows) + shipped tiles
            aD = np.asarray(r["accD"], np.float32)         # [128, 4096]
            colmin = aD.min(axis=0)                        # [4096]
            # reduce each unit to (best value, candidate col pair) per row
            bv = np.empty((2, T_TILES, 128), np.float32)
            bc = np.empty((2, T_TILES, 128, 2), np.int64)
            for u in range(N_UNITS):
                h, t = divmod(u, T_TILES)
                base = h * HALF
                if shipped[u]:
                    j = np.argmin(cm[u], axis=1)           # [128]
                    bv[h, t] = cm[u][np.arange(128), j]
                    bc[h, t, :, 0] = base + j
                    bc[h, t, :, 1] = base + j
                    np.minimum(colmin[base:base + HALF], cm[u].min(axis=0),
                               out=colmin[base:base + HALF])
                else:
                    M = cm[u, :, 0:1024]
                    j = np.argmin(M, axis=1)
                    bv[h, t] = M[np.arange(128), j]
                    bc[h, t, :, 0] = base + j
                    bc[h, t, :, 1] = base + j + 1024
            col_partials.append(colmin)
            # pick the winning half per row, recompute its <=2 candidates
            hwin = np.argmin(bv, axis=0).reshape(ROWS_PER_CORE)   # [2048]
            cand = bc.transpose(1, 2, 0, 3).reshape(ROWS_PER_CORE, 2, 2)
            cand = cand[rows_l, hwin]                      # [2048, 2]

            xb = pred_pts[b, rh * ROWS_PER_CORE:(rh + 1) * ROWS_PER_CORE]
            ycand = gt_pts[b][cand]                        # [2048, 2, 3]
            d2c = (
                x2_all[b, rh * ROWS_PER_CORE:(rh + 1) * ROWS_PER_CORE][:, None]
                + y2_all[b][cand]
                - 2.0 * np.einsum("nd,nkd->nk", xb, ycand, dtype=np.float32)
            ).astype(np.float32)
            d2c = np.maximum(d2c, 0.0)
            within = np.argmin(d2c, axis=1)
            sl = slice(rh * ROWS_PER_CORE, (rh + 1) * ROWS_PER_CORE)
            min_x2y[b, sl] = d2c[rows_l, within]
            idx_p2g[b, sl] = cand[rows_l, within]

        acc_b = np.minimum(col_partials[0], col_partials[1])
        min_y2x[b] = np.maximum(acc_b, 0.0)

    chamfer = np.float32(np.mean(min_x2y)) + np.float32(np.mean(min_y2x))

    # normal consistency
    matched = np.take_along_axis(gt_nrm, idx_p2g[..., None], axis=1)
    cos = np.abs(np.sum(pred_nrm * matched, axis=-1))
    normal_loss = np.float32(np.mean(1.0 - cos))

    # edge loss (exact, on host)
    pf = np.asarray(predicted_faces).astype(np.int64)
    v0 = predicted_vertices[:, pf[:, 0]]
    v1 = predicted_vertices[:, pf[:, 1]]
    v2 = predicted_vertices[:, pf[:, 2]]
    e = np.concatenate([v1 - v0, v2 - v1, v0 - v2], axis=1)
    edge_loss = np.float32(np.mean(np.sum(e * e, axis=-1)))

    total = (
        np.float32(CHAMFER_W) * chamfer
        + np.float32(NORM_W) * normal_loss
        + np.float32(EDGE_W) * edge_loss
    )
    return np.asarray(total, dtype=np.float32)
